# revision 25
# baseline (speedup 1.0000x reference)
"""DirectionalConv3d Trainium2 kernel — pack-T2 layout, bf16 I/O.

out[b, o, t, r, c] = sum_d W_d[o, :] . x[b, :, (t,r,c)+delta_d]
for the 7-point directional stencil (self, t+-1, r+-1, c+-1), zero padded.

Strategy (1 batch per core, 8 cores):
  - Host casts x to bf16 and packs plane-parity onto partition halves:
    partition p<64 holds channel p of EVEN t-planes, partition 64+p holds
    channel p of ODD t-planes ("superplane" u = plane pair (2u, 2u+1),
    free dim = u*1024 + r*32 + c).  Output uses the same packed layout
    (psum partitions 0-63 = out[2u] channels, 64-127 = out[2u+1]) and is
    written back as bf16; the host unpacks and casts to f32.
  - The PE is the bottleneck (measured: bf16 streams 2 cols/cycle for
    K=64 quadrant matmuls, 1 col/cycle for K=128 — both 16384 MAC/cycle,
    so only weight-block DENSITY buys anything).  The 14 direction
    instances (7 stencil taps x 2 plane parities) are covered by:
      * dense pass A at rhs offset 0: lhsT=[[Wself,Wtp],[Wtm,Wself]] —
        self(both) + tp/tm internal to the pair;
      * dense pass D at rhs offset +32 against a shifted image copy
        (odd half stored at position+1056, psum odd half at position+992):
        rm-even + tp-even-cross + tm-odd-cross + rp-odd in ONE full-
        density matmul.  Its psum tile is combined with the natural one
        during evacuation (ACT copies the shifted tile into bf16 staging,
        DVE adds the natural tile on top);
      * c+-1 as block-diagonal K=128 matmuls with 2-D strided APs
        [16 rows, 31 cols] (no padding, no wrap corrections);
      * the leftovers rp-even / rm-odd as K=64 quadrants (2 cols/cycle),
        plus two tiny N=32 row-boundary correction matmuls with negated
        weights (pass D cannot row-trim per-instance), and an rp quadrant
        for out-odd[15] whose shifted psum bank does not exist.
  - The shifted image copy: even half re-read from HBM at +32 (SP ring),
    odd half built by the otherwise-idle GpSimd from the resident chunks.
  - Three DMA streams on three independent rings (SWDGE=input-lo,
    SP=input-hi + shifted-even, ACT=output) since each HWDGE ring drains
    strictly FIFO.  Dummy warm-up matmuls hold the PE clock at 2.4 GHz
    through the fill phase.
"""

import numpy as np
import ml_dtypes
import os

B = 8
CI = 64
CO = 64
T = 32
R = 32
C = 32
U = T // 2           # 16 superplanes
SPL = R * C          # 1024 elements per (super)plane per partition
NF = U * SPL         # 16384 free elements per partition half
WCOLS = 704          # weight SBUF columns: A | rp | rm | cp | cm | cross

SG = int(os.environ.get("KERNEL_SG", "2"))    # superplanes per output stage
# input chunk sizes in superplanes: small first chunks let the first
# matmul start as soon as ~256 KB has landed instead of ~1 MB.
CHUNKS = [int(c) for c in os.environ.get("KERNEL_CHUNKS", "1,1,2,4,4,4").split(",")]
assert sum(CHUNKS) == U
_CHUNK_OF = []
for _k, _c in enumerate(CHUNKS):
    _CHUNK_OF += [_k] * _c
_CHUNK_BASE = [sum(CHUNKS[:k]) for k in range(len(CHUNKS))]

# output stage groups: SG superplanes each, but the last two flush singly
_SGROUPS = []
_u = 0
while _u < U:
    _g1 = min(_u + SG, U) if _u < U - 2 else _u + 1
    _SGROUPS.append((_u, _g1))
    _u = _g1
_SGROUP_OF = {}
for _g0, _g1 in _SGROUPS:
    for _uu in range(_g0, _g1):
        _SGROUP_OF[_uu] = (_g0, _g1)

_NC_CACHE = {}


def _emit(nc, tc, x, wt, out, mybir, bass):
    xdt = mybir.dt.bfloat16
    AP = bass.AP

    wpool = tc.alloc_tile_pool(name="wp", bufs=1)
    xpool = tc.alloc_tile_pool(name="xin", bufs=1)
    x2pool = tc.alloc_tile_pool(name="x2in", bufs=1)
    apool = tc.alloc_tile_pool(name="accp", bufs=8, space="PSUM")
    spool = tc.alloc_tile_pool(name="stg", bufs=4)

    # ---- weights [128, 704] prepacked host-side (see host_weights) ----
    w_sb = wpool.tile([128, WCOLS], xdt, name="w_sb")
    nc.sync.dma_start(out=w_sb[0:64, :], in_=AP(wt.tensor, 0, [[WCOLS, 64], [1, WCOLS]]))
    nc.sync.dma_start(out=w_sb[64:128, :],
                      in_=AP(wt.tensor, 64 * WCOLS, [[WCOLS, 64], [1, WCOLS]]))
    wA = w_sb[:, 0:128]            # dense pass 1: self + internal tp/tm
    wD = w_sb[:, 128:256]          # dense pass 2: rm-e + tp-e + tm-o + rp-o
    wCP = w_sb[:, 256:384]
    wCM = w_sb[:, 384:512]
    wLQ = w_sb[:, 512:576]         # leftover quadrants: lo=Wrp (rp-e), hi=Wrm (rm-o)
    wCR = w_sb[:, 576:640]         # corrections: lo=-Wrm, hi=-Wrp
    wR15 = w_sb[64:128, 640:704]   # rp for out-odd[15] (quadrant 64,64)

    # ---- x image: direct bf16 DMA, no staging, no casts, no memsets ----
    # two 64-partition DMAs per chunk (complementary SDMA engine sets);
    # lo half on SWDGE (gpsimd — its own descriptor path, dodges the
    # HWDGE rings that are busy with the runtime ACT-table preamble),
    # hi half on the SP HWDGE ring.  Descriptor generation runs on two
    # independent engines this way.
    xts = []
    x2ts = []
    for k, c in enumerate(CHUNKS):
        n = c * SPL
        off = _CHUNK_BASE[k] * SPL
        xt = xpool.tile([128, n], xdt, name=f"xc{k}")
        nc.gpsimd.dma_start(out=xt[0:64, :],
                            in_=AP(x.tensor, off, [[NF, 64], [1, n]]))
        nc.sync.dma_start(out=xt[64:128, :],
                          in_=AP(x.tensor, 64 * NF + off, [[NF, 64], [1, n]]))
        xts.append(xt)
        # xt2: the +32-phase image for dense pass 2.  Even half = even
        # planes at positions [off+32, off+n+32) (straight HBM re-read,
        # interleaved on the SP ring so chunk k is complete before chunk
        # k+1 starts); odd half = odd planes at positions [off-1024,
        # off+n-1024) (built later by GpSimd copies from the xts chunks —
        # zero extra HBM traffic; the <0 head region is the zero guard
        # that makes tp/rp at the t=0 boundary come out right for free).
        x2 = x2pool.tile([128, n], xdt, name=f"x2c{k}")
        ne = min(n, NF - (off + 32))
        nc.sync.dma_start(out=x2[0:64, :ne],
                          in_=AP(x.tensor, off + 32, [[NF, 64], [1, ne]]))
        if ne < n:
            nc.vector.memset(x2[0:64, ne:n], 0.0)
        x2ts.append(x2)

    # xt2 odd halves (GpSimd: otherwise-idle engine; desc-gen for the
    # input DMAs above is already queued ahead of these on the Q7).
    for k, c in enumerate(CHUNKS):
        n = c * SPL
        lo = _CHUNK_BASE[k] * SPL - SPL       # odd positions [lo, lo+n)
        if lo < 0:
            nc.gpsimd.memset(x2ts[k][64:128, 0:-lo], 0.0)
        for h, ch in enumerate(CHUNKS):
            hb = _CHUNK_BASE[h] * SPL
            i0 = max(lo, hb)
            i1 = min(lo + n, hb + ch * SPL)
            if i0 < i1:
                nc.gpsimd.tensor_copy(
                    out=x2ts[k][64:128, i0 - lo:i1 - lo],
                    in_=xts[h][64:128, i0 - hb:i1 - hb])

    def xv(u, lo, sz, p0=0, p1=128):
        """SBUF AP for packed superplane u, free offset lo, length sz."""
        k = _CHUNK_OF[u]
        base = (u - _CHUNK_BASE[k]) * SPL
        return xts[k][p0:p1, base + lo:base + lo + sz]

    def x2v(u, j):
        """xt2 AP for dense pass 2 of superplane u, bank j (512 wide)."""
        k = _CHUNK_OF[u]
        base = (u - _CHUNK_BASE[k]) * SPL + j * 512
        return x2ts[k][:, base:base + 512]

    def xvr(u, j, p0=0, p1=128):
        """[p, 16 rows, 32 cols] view of bank j's rows of superplane u."""
        k = _CHUNK_OF[u]
        base = (u - _CHUNK_BASE[k]) * SPL + j * 512
        return xts[k][p0:p1, base:base + 512].rearrange("p (r c) -> p r c", c=C)

    mm = nc.tensor.matmul

    # ---- PE warm-up: dummy matmuls on a locally-memset tile keep the HAM
    # activity monitor busy while the x chunks stream in, so the real
    # matmuls start at 2.4 GHz instead of ramping from 1.2.  PE would
    # otherwise idle through the whole fill phase.  Deliberately NOT on
    # w_sb: a DMA dependency would gate the warm-up on semaphores that
    # fire ~12us in (measured), defeating the point.
    nwarm = int(os.environ.get("KERNEL_WARM", "56"))
    if nwarm:
        wsrc = wpool.tile([128, 128], xdt, name="wsrc")
        nc.vector.memset(wsrc[:, :], 0.0)
        wacc = apool.tile([128, 128], mybir.dt.float32, name="wacc", tag="acc")
        for i in range(nwarm):
            mm(out=wacc[:, :], lhsT=wsrc[:, :], rhs=wsrc[:, :],
               start=True, stop=True, skip_group_check=True)

    accB = [None] * U   # natural accumulation (A, c+-, leftovers, corrs)
    accA = [None] * U   # dense-pass-2 accumulation (psum addr = pos + 992
    #                     for the odd half: out-odd[q] lives at addr q+992)
    stage_ref = [None]

    def emit_evac(v):
        """Combine accA/accB into bf16 staging and DMA per stage group.
        out-even[v] = accB[v].lo + accA[v].lo (aligned);
        out-odd[v]  = accB[v].hi + accA segments at addr q+992:
          q-local [0,32) -> accA[v][1] local [480,512)
          [32,544)       -> accA[v+1][0]
          [544,1024)     -> accA[v+1][1] local [0,480)   (v<15 only).
        Step 1: ACT copies the accA parts into stage (casts to bf16);
        step 2: DVE adds accB on top (1 psum + 1 sbuf read = full rate)."""
        g0, g1 = _SGROUP_OF[v]
        if v == g0:
            stage_ref[0] = spool.tile([128, (g1 - g0) * SPL], xdt,
                                      name=f"st{v}", tag="st")
        st = stage_ref[0]
        so = (v - g0) * SPL
        nc.scalar.copy(out=st[0:64, so:so + 512], in_=accA[v][0][0:64, :])
        nc.scalar.copy(out=st[0:64, so + 512:so + SPL], in_=accA[v][1][0:64, :])
        nc.scalar.copy(out=st[64:128, so:so + 32], in_=accA[v][1][64:128, 480:512])
        if v < U - 1:
            nc.scalar.copy(out=st[64:128, so + 32:so + 544],
                           in_=accA[v + 1][0][64:128, :])
            nc.scalar.copy(out=st[64:128, so + 544:so + SPL],
                           in_=accA[v + 1][1][64:128, 0:480])
            for j in range(2):
                nc.vector.tensor_add(st[:, so + j * 512:so + (j + 1) * 512],
                                     st[:, so + j * 512:so + (j + 1) * 512],
                                     accB[v][j][:, :])
        else:
            # no accA[v+1]: out-odd[15] rows>=1 live in accB only
            for j in range(2):
                nc.vector.tensor_add(st[0:64, so + j * 512:so + (j + 1) * 512],
                                     st[0:64, so + j * 512:so + (j + 1) * 512],
                                     accB[v][j][0:64, :])
            nc.vector.tensor_add(st[64:128, so:so + 32],
                                 st[64:128, so:so + 32],
                                 accB[v][0][64:128, 0:32])
            nc.vector.tensor_copy(out=st[64:128, so + 32:so + 512],
                                  in_=accB[v][0][64:128, 32:512])
            nc.vector.tensor_copy(out=st[64:128, so + 512:so + SPL],
                                  in_=accB[v][1][64:128, :])
        if v == g1 - 1:
            # out on the ACT HWDGE ring: SP drains input-hi + xt2-even,
            # SWDGE drains input-lo.
            n = (g1 - g0) * SPL
            nc.scalar.dma_start(
                out=AP(out.tensor, g0 * SPL, [[NF, 64], [1, n]]),
                in_=st[0:64, :n])
            nc.scalar.dma_start(
                out=AP(out.tensor, 64 * NF + g0 * SPL, [[NF, 64], [1, n]]),
                in_=st[64:128, :n])

    for u in range(U):
        accB[u] = [apool.tile([128, 512], mybir.dt.float32, name=f"b{u}_{j}",
                              tag="acc") for j in range(2)]
        accA[u] = [apool.tile([128, 512], mybir.dt.float32, name=f"d{u}_{j}",
                              tag="acc") for j in range(2)]
        # --- K=128 full-array matmuls (LDWEIGHTS hides in the background
        # weight buffer): two dense passes + the c+-1 block-diagonals.
        for j in range(2):  # A: self(both) + tp/tm internal (dense)
            mm(out=accB[u][j][:, :], lhsT=wA, rhs=xv(u, j * 512, 512),
               start=True, stop=False, skip_group_check=True)
        for j in range(2):  # dense pass 2: rm-e + tp-e + tm-o + rp-o
            mm(out=accA[u][j][:, :], lhsT=wD, rhs=x2v(u, j),
               start=True, stop=True, skip_group_check=True)
        ovs = [accB[u][j][:, :].rearrange("p (r c) -> p r c", c=C)
               for j in range(2)]
        xrs = [xvr(u, j) for j in range(2)]
        for j in range(2):
            mm(out=ovs[j][:, :, 1:32], lhsT=wCP, rhs=xrs[j][:, :, 0:31],
               start=False, stop=False, skip_group_check=True)
        for j in range(2):
            mm(out=ovs[j][:, :, 0:31], lhsT=wCM, rhs=xrs[j][:, :, 1:32],
               start=False, stop=False, skip_group_check=True)
        # --- K=64 quadrant group (2-col/cycle): rp-e, rm-o leftovers,
        # the row-boundary corrections for dense pass 2, and the rp
        # contribution for out-odd[15] that has no accA bank.
        mm(out=accB[u][0][0:64, 32:512], lhsT=wLQ[0:64, :], rhs=xv(u, 0, 480, 0, 64),
           start=False, stop=False, skip_group_check=True)
        mm(out=accB[u][1][0:64, 0:512], lhsT=wLQ[0:64, :], rhs=xv(u, 480, 512, 0, 64),
           start=False, stop=False, skip_group_check=True)
        mm(out=accB[u][0][64:128, 0:512], lhsT=wLQ[64:128, :],
           rhs=xv(u, 32, 512, 64, 128),
           start=False, stop=(u == 0), skip_group_check=True)
        mm(out=accB[u][1][64:128, 0:480], lhsT=wLQ[64:128, :],
           rhs=xv(u, 544, 480, 64, 128),
           start=False, stop=False, skip_group_check=True)
        if u < U - 1:  # rm-e row-31 correction: -Wrm . x_even[next plane row 0]
            mm(out=accB[u][1][0:64, 480:512], lhsT=wCR[0:64, :],
               rhs=xv(u + 1, 0, 32, 0, 64),
               start=False, stop=True, skip_group_check=True)
        if u > 0:      # rp-o row-0 correction: -Wrp . x_odd[prev plane row 31]
            mm(out=accB[u][0][64:128, 0:32], lhsT=wCR[64:128, :],
               rhs=xv(u - 1, 992, 32, 64, 128),
               start=False, stop=(u < U - 1), skip_group_check=True)
        if u == U - 1:  # rp for out-odd[15] rows 1-31 (accA has no bank here)
            mm(out=accB[u][0][64:128, 32:512], lhsT=wR15,
               rhs=xv(u, 0, 480, 64, 128),
               start=False, stop=True, skip_group_check=True)
            mm(out=accB[u][1][64:128, 0:512], lhsT=wR15,
               rhs=xv(u, 480, 512, 64, 128),
               start=False, stop=True, skip_group_check=True)

        if u >= 1:
            emit_evac(u - 1)
    emit_evac(U - 1)

    for p in (spool, apool, x2pool, xpool, wpool):
        p.release()


def _split_multi_waits(nc, mybir):
    """Walrus codegen allows only one sem-wait slot per engine instruction
    ("Too many sync wait commands").  Hoist all but one wait of any
    multi-wait instruction onto InstNoOp's inserted immediately before it
    on the same engine queue — semantically identical for in-order
    engines (the nop blocks the queue until its wait passes).
    """
    SyncInfo = mybir.SyncInfo
    counter = [0]
    for blk in nc.m.functions[0].blocks:
        insts = list(blk.instructions)
        out, changed = [], False
        for inst in insts:
            si = getattr(inst, "sync_info", None)
            waits = list(si.on_wait) if si is not None and si.on_wait else []
            if len(waits) > 1:
                for w in waits[:-1]:
                    nop = mybir.InstNoOp(name=f"waitnop_{counter[0]}")
                    counter[0] += 1
                    nop.engine = inst.engine
                    nop.sync_info = SyncInfo(on_wait=[w], on_update=[])
                    nc.register_instruction(nop, overwrite=True)
                    out.append(nop)
                si.on_wait = [waits[-1]]
                changed = True
            out.append(inst)
        if changed:
            blk.instructions = out


def build_nc():
    import concourse.bass as bass
    import concourse.mybir as mybir
    import concourse.tile as tile

    key = (tuple(CHUNKS), SG)
    if key in _NC_CACHE:
        return _NC_CACHE[key]
    nc = bass.Bass("TRN2", target_bir_lowering=False, debug=False)
    x = nc.dram_tensor("x", [128, NF], mybir.dt.bfloat16, kind="ExternalInput").ap()
    wt = nc.dram_tensor("wt", [128, WCOLS], mybir.dt.bfloat16,
                        kind="ExternalInput").ap()
    out = nc.dram_tensor("out", [128, NF], mybir.dt.bfloat16,
                         kind="ExternalOutput").ap()
    with tile.TileContext(nc) as tc:
        _emit(nc, tc, x, wt, out, mybir, bass)
    _split_multi_waits(nc, mybir)
    _NC_CACHE[key] = nc
    return nc


def host_weights(inputs):
    """Pack the 7 64x64 weights into the [128, 704] bf16 lhsT block layout.

    cols 0-127:   A  = [[Wself^T, Wtp^T], [Wtm^T, Wself^T]]   (dense pass 1)
    cols 128-255: wD = [[Wrm^T,  Wtm^T], [Wtp^T, Wrp^T]]      (dense pass 2)
    cols 256-511: diag2(Wcp), diag2(Wcm)
    cols 512-575: lo = Wrp^T (rp-e quadrant), hi = Wrm^T (rm-o quadrant)
    cols 576-639: lo = -Wrm^T, hi = -Wrp^T (pass-2 row-boundary corrections)
    cols 640-703: hi = Wrp^T (out-odd[15] rp quadrant)
    """
    W = {n: np.asarray(inputs[n], dtype=np.float32)
         for n in ("w_self", "w_tp", "w_tm", "w_rp", "w_rm", "w_cp", "w_cm")}
    wt = np.zeros((128, WCOLS), np.float32)
    wt[0:64, 0:64] = W["w_self"].T
    wt[64:128, 0:64] = W["w_tm"].T
    wt[0:64, 64:128] = W["w_tp"].T
    wt[64:128, 64:128] = W["w_self"].T
    wt[0:64, 128:192] = W["w_rm"].T
    wt[64:128, 128:192] = W["w_tp"].T
    wt[0:64, 192:256] = W["w_tm"].T
    wt[64:128, 192:256] = W["w_rp"].T
    for i, n in enumerate(("w_cp", "w_cm")):
        c0 = 256 + i * 128
        wt[0:64, c0:c0 + 64] = W[n].T
        wt[64:128, c0 + 64:c0 + 128] = W[n].T
    wt[0:64, 512:576] = W["w_rp"].T
    wt[64:128, 512:576] = W["w_rm"].T
    wt[0:64, 576:640] = -W["w_rm"].T
    wt[64:128, 576:640] = -W["w_rp"].T
    wt[64:128, 640:704] = W["w_rp"].T
    return wt.astype(ml_dtypes.bfloat16)


def host_x(inputs):
    """Per-batch packed bf16 images [128, 16384]: even planes on rows 0-63,
    odd planes on rows 64-127."""
    x = np.asarray(inputs["x"], dtype=np.float32)
    xs = []
    for b in range(B):
        xe = x[b][:, 0::2].reshape(CI, NF)
        xo = x[b][:, 1::2].reshape(CI, NF)
        xs.append(np.ascontiguousarray(
            np.concatenate([xe, xo], axis=0)).astype(ml_dtypes.bfloat16))
    return xs


def host_out(res):
    """Unpack per-core [128, 16384] bf16 results to [B, 64, 32, 32, 32] f32."""
    out = np.empty((B, CO, T, R, C), np.float32)
    for b in range(B):
        o = np.asarray(res[b]["out"]).astype(np.float32).reshape(2, CO, U, R, C)
        out[b, :, 0::2] = o[0]
        out[b, :, 1::2] = o[1]
    return out


def kernel(**inputs):
    from concourse.bass_utils import run_bass_kernel_spmd

    nc = build_nc()
    wt = host_weights(inputs)
    xs = host_x(inputs)
    in_maps = [{"x": xs[b], "wt": wt} for b in range(B)]
    res = run_bass_kernel_spmd(nc, in_maps, list(range(B))).results
    return host_out(res)


# revision 28
# speedup vs baseline: 1.0583x; 1.0583x over previous
"""DirectionalConv3d Trainium2 kernel — pack-T2 layout, bf16 I/O.

out[b, o, t, r, c] = sum_d W_d[o, :] . x[b, :, (t,r,c)+delta_d]
for the 7-point directional stencil (self, t+-1, r+-1, c+-1), zero padded.

Strategy (1 batch per core, 8 cores):
  - Host casts x to bf16 and packs plane-parity onto partition halves:
    partition p<64 holds channel p of EVEN t-planes, partition 64+p holds
    channel p of ODD t-planes ("superplane" u = plane pair (2u, 2u+1),
    free dim = u*1024 + r*32 + c).  Output uses the same packed layout
    (psum partitions 0-63 = out[2u] channels, 64-127 = out[2u+1]) and is
    written back as bf16; the host unpacks and casts to f32.
  - The PE is the bottleneck (measured: bf16 streams 2 cols/cycle for
    K=64 quadrant matmuls, 1 col/cycle for K=128 — both 16384 MAC/cycle,
    so only weight-block DENSITY buys anything).  The 14 direction
    instances (7 stencil taps x 2 plane parities) are covered by:
      * dense pass A at rhs offset 0: lhsT=[[Wself,Wtp],[Wtm,Wself]] —
        self(both) + tp/tm internal to the pair;
      * dense pass D at rhs offset +32 against a shifted image copy
        (odd half stored at position+1056, psum odd half at position+992):
        rm-even + tp-even-cross + tm-odd-cross + rp-odd in ONE full-
        density matmul.  Its psum tile is combined with the natural one
        during evacuation (ACT copies the shifted tile into bf16 staging,
        DVE adds the natural tile on top);
      * c+-1 as block-diagonal K=128 matmuls with 2-D strided APs
        [16 rows, 31 cols] (no padding, no wrap corrections);
      * the leftovers rp-even / rm-odd as K=64 quadrants (2 cols/cycle),
        plus two tiny N=32 row-boundary correction matmuls with negated
        weights (pass D cannot row-trim per-instance), and an rp quadrant
        for out-odd[15] whose shifted psum bank does not exist.
  - The shifted image copy: even half re-read from HBM at +32 (SP ring),
    odd half built by the otherwise-idle GpSimd from the resident chunks.
  - Three DMA streams on three independent rings (SWDGE=input-lo,
    SP=input-hi + shifted-even, ACT=output) since each HWDGE ring drains
    strictly FIFO.  Dummy warm-up matmuls hold the PE clock at 2.4 GHz
    through the fill phase.
"""

import numpy as np
import ml_dtypes
import os

B = 8
CI = 64
CO = 64
T = 32
R = 32
C = 32
U = T // 2           # 16 superplanes
SPL = R * C          # 1024 elements per (super)plane per partition
NF = U * SPL         # 16384 free elements per partition half
WCOLS = 704          # weight SBUF columns: A | rp | rm | cp | cm | cross

SG = int(os.environ.get("KERNEL_SG", "2"))    # superplanes per output stage
# input chunk sizes in superplanes: small first chunks let the first
# matmul start as soon as ~256 KB has landed instead of ~1 MB.
CHUNKS = [int(c) for c in os.environ.get("KERNEL_CHUNKS", "1,1,2,4,4,4").split(",")]
assert sum(CHUNKS) == U
_CHUNK_OF = []
for _k, _c in enumerate(CHUNKS):
    _CHUNK_OF += [_k] * _c
_CHUNK_BASE = [sum(CHUNKS[:k]) for k in range(len(CHUNKS))]

# output stage groups: SG superplanes each, but the last two flush singly
_SGROUPS = []
_u = 0
while _u < U:
    _g1 = min(_u + SG, U) if _u < U - 2 else _u + 1
    _SGROUPS.append((_u, _g1))
    _u = _g1
_SGROUP_OF = {}
for _g0, _g1 in _SGROUPS:
    for _uu in range(_g0, _g1):
        _SGROUP_OF[_uu] = (_g0, _g1)

_NC_CACHE = {}


def _emit(nc, tc, x, wt, out, mybir, bass):
    xdt = mybir.dt.bfloat16
    AP = bass.AP

    wpool = tc.alloc_tile_pool(name="wp", bufs=1)
    xpool = tc.alloc_tile_pool(name="xin", bufs=1)
    x2pool = tc.alloc_tile_pool(name="x2in", bufs=1)
    apool = tc.alloc_tile_pool(name="accp", bufs=8, space="PSUM")
    spool = tc.alloc_tile_pool(name="stg", bufs=4)

    # ---- weights [128, 704] prepacked host-side (see host_weights) ----
    w_sb = wpool.tile([128, WCOLS], xdt, name="w_sb")
    nc.sync.dma_start(out=w_sb[0:64, :], in_=AP(wt.tensor, 0, [[WCOLS, 64], [1, WCOLS]]))
    nc.sync.dma_start(out=w_sb[64:128, :],
                      in_=AP(wt.tensor, 64 * WCOLS, [[WCOLS, 64], [1, WCOLS]]))
    wA = w_sb[:, 0:128]            # dense pass 1: self + internal tp/tm
    wD = w_sb[:, 128:256]          # dense pass 2: rm-e + tp-e + tm-o + rp-o
    wCP = w_sb[:, 256:384]
    wCM = w_sb[:, 384:512]
    wLQ = w_sb[:, 512:576]         # leftover quadrants: lo=Wrp (rp-e), hi=Wrm (rm-o)
    wCR = w_sb[:, 576:640]         # corrections: lo=-Wrm, hi=-Wrp
    wR15 = w_sb[64:128, 640:704]   # rp for out-odd[15] (quadrant 64,64)

    # ---- x image: direct bf16 DMA, no staging, no casts, no memsets ----
    # two 64-partition DMAs per chunk (complementary SDMA engine sets);
    # lo half on SWDGE (gpsimd — its own descriptor path, dodges the
    # HWDGE rings that are busy with the runtime ACT-table preamble),
    # hi half on the SP HWDGE ring.  Descriptor generation runs on two
    # independent engines this way.
    xts = []
    x2ts = []
    for k, c in enumerate(CHUNKS):
        n = c * SPL
        off = _CHUNK_BASE[k] * SPL
        xt = xpool.tile([128, n], xdt, name=f"xc{k}")
        nc.gpsimd.dma_start(out=xt[0:64, :],
                            in_=AP(x.tensor, off, [[NF, 64], [1, n]]))
        # xt2 odd half (positions [off-1024, off+n-1024)): direct HBM
        # re-read, interleaved per chunk on the SWDGE ring (FIFO per ring
        # -- emitting all input-lo chunks first would head-of-line block
        # the shifted halves and stall dense pass 2).  Chunk 0's range is
        # entirely before t=0 -> zero guard via memset.
        x2 = x2pool.tile([128, n], xdt, name=f"x2c{k}")
        lo = off - SPL
        if lo < 0:
            nc.vector.memset(x2[64:128, 0:min(-lo, n)], 0.0)
        if lo + n > 0:
            d0 = max(0, -lo)
            nc.gpsimd.dma_start(
                out=x2[64:128, d0:n],
                in_=AP(x.tensor, 64 * NF + lo + d0, [[NF, 64], [1, n - d0]]))
        nc.sync.dma_start(out=xt[64:128, :],
                          in_=AP(x.tensor, 64 * NF + off, [[NF, 64], [1, n]]))
        xts.append(xt)
        # xt2 even half: even planes at positions [off+32, off+n+32)
        # (straight HBM re-read on the SP ring, interleaved per chunk;
        # the +32 phase gives dense pass 2 its shifted view.  A GpSimd Q7
        # copy was tried for the shifted halves first: 52us for 4 MB,
        # ~3.5x slower than DVE -- don't).
        ne = min(n, NF - (off + 32))
        nc.sync.dma_start(out=x2[0:64, :ne],
                          in_=AP(x.tensor, off + 32, [[NF, 64], [1, ne]]))
        if ne < n:
            nc.vector.memset(x2[0:64, ne:n], 0.0)
        x2ts.append(x2)

    def xv(u, lo, sz, p0=0, p1=128):
        """SBUF AP for packed superplane u, free offset lo, length sz."""
        k = _CHUNK_OF[u]
        base = (u - _CHUNK_BASE[k]) * SPL
        return xts[k][p0:p1, base + lo:base + lo + sz]

    def x2v(u, j):
        """xt2 AP for dense pass 2 of superplane u, bank j (512 wide)."""
        k = _CHUNK_OF[u]
        base = (u - _CHUNK_BASE[k]) * SPL + j * 512
        return x2ts[k][:, base:base + 512]

    def xvr(u, j, p0=0, p1=128):
        """[p, 16 rows, 32 cols] view of bank j's rows of superplane u."""
        k = _CHUNK_OF[u]
        base = (u - _CHUNK_BASE[k]) * SPL + j * 512
        return xts[k][p0:p1, base:base + 512].rearrange("p (r c) -> p r c", c=C)

    mm = nc.tensor.matmul

    # ---- PE warm-up: dummy matmuls on a locally-memset tile keep the HAM
    # activity monitor busy while the x chunks stream in, so the real
    # matmuls start at 2.4 GHz instead of ramping from 1.2.  PE would
    # otherwise idle through the whole fill phase.  Deliberately NOT on
    # w_sb: a DMA dependency would gate the warm-up on semaphores that
    # fire ~12us in (measured), defeating the point.
    nwarm = int(os.environ.get("KERNEL_WARM", "56"))
    if nwarm:
        wsrc = wpool.tile([128, 128], xdt, name="wsrc")
        nc.vector.memset(wsrc[:, :], 0.0)
        wacc = apool.tile([128, 128], mybir.dt.float32, name="wacc", tag="acc")
        for i in range(nwarm):
            mm(out=wacc[:, :], lhsT=wsrc[:, :], rhs=wsrc[:, :],
               start=True, stop=True, skip_group_check=True)

    accB = [None] * U   # natural accumulation (A, c+-, leftovers, corrs)
    accA = [None] * U   # dense-pass-2 accumulation (psum addr = pos + 992
    #                     for the odd half: out-odd[q] lives at addr q+992)
    stage_ref = [None]

    def emit_evac(v):
        """Combine accA/accB into bf16 staging and DMA per stage group.
        out-even[v] = accB[v].lo + accA[v].lo (aligned);
        out-odd[v]  = accB[v].hi + accA segments at addr q+992:
          q-local [0,32) -> accA[v][1] local [480,512)
          [32,544)       -> accA[v+1][0]
          [544,1024)     -> accA[v+1][1] local [0,480)   (v<15 only).
        Step 1: ACT copies the accA parts into stage (casts to bf16);
        step 2: DVE adds accB on top (1 psum + 1 sbuf read = full rate)."""
        g0, g1 = _SGROUP_OF[v]
        if v == g0:
            stage_ref[0] = spool.tile([128, (g1 - g0) * SPL], xdt,
                                      name=f"st{v}", tag="st")
        st = stage_ref[0]
        so = (v - g0) * SPL
        nc.scalar.copy(out=st[0:64, so:so + 512], in_=accA[v][0][0:64, :])
        nc.scalar.copy(out=st[0:64, so + 512:so + SPL], in_=accA[v][1][0:64, :])
        nc.scalar.copy(out=st[64:128, so:so + 32], in_=accA[v][1][64:128, 480:512])
        if v < U - 1:
            nc.scalar.copy(out=st[64:128, so + 32:so + 544],
                           in_=accA[v + 1][0][64:128, :])
            nc.scalar.copy(out=st[64:128, so + 544:so + SPL],
                           in_=accA[v + 1][1][64:128, 0:480])
            for j in range(2):
                nc.vector.tensor_add(st[:, so + j * 512:so + (j + 1) * 512],
                                     st[:, so + j * 512:so + (j + 1) * 512],
                                     accB[v][j][:, :])
        else:
            # no accA[v+1]: out-odd[15] rows>=1 live in accB only
            for j in range(2):
                nc.vector.tensor_add(st[0:64, so + j * 512:so + (j + 1) * 512],
                                     st[0:64, so + j * 512:so + (j + 1) * 512],
                                     accB[v][j][0:64, :])
            nc.vector.tensor_add(st[64:128, so:so + 32],
                                 st[64:128, so:so + 32],
                                 accB[v][0][64:128, 0:32])
            nc.vector.tensor_copy(out=st[64:128, so + 32:so + 512],
                                  in_=accB[v][0][64:128, 32:512])
            nc.vector.tensor_copy(out=st[64:128, so + 512:so + SPL],
                                  in_=accB[v][1][64:128, :])
        if v == g1 - 1:
            # out on the ACT HWDGE ring: SP drains input-hi + xt2-even,
            # SWDGE drains input-lo.
            n = (g1 - g0) * SPL
            nc.scalar.dma_start(
                out=AP(out.tensor, g0 * SPL, [[NF, 64], [1, n]]),
                in_=st[0:64, :n])
            nc.scalar.dma_start(
                out=AP(out.tensor, 64 * NF + g0 * SPL, [[NF, 64], [1, n]]),
                in_=st[64:128, :n])

    for u in range(U):
        accB[u] = [apool.tile([128, 512], mybir.dt.float32, name=f"b{u}_{j}",
                              tag="acc") for j in range(2)]
        accA[u] = [apool.tile([128, 512], mybir.dt.float32, name=f"d{u}_{j}",
                              tag="acc") for j in range(2)]
        # --- K=128 full-array matmuls (LDWEIGHTS hides in the background
        # weight buffer): two dense passes + the c+-1 block-diagonals.
        for j in range(2):  # A: self(both) + tp/tm internal (dense)
            mm(out=accB[u][j][:, :], lhsT=wA, rhs=xv(u, j * 512, 512),
               start=True, stop=False, skip_group_check=True)
        for j in range(2):  # dense pass 2: rm-e + tp-e + tm-o + rp-o
            mm(out=accA[u][j][:, :], lhsT=wD, rhs=x2v(u, j),
               start=True, stop=True, skip_group_check=True)
        ovs = [accB[u][j][:, :].rearrange("p (r c) -> p r c", c=C)
               for j in range(2)]
        xrs = [xvr(u, j) for j in range(2)]
        for j in range(2):
            mm(out=ovs[j][:, :, 1:32], lhsT=wCP, rhs=xrs[j][:, :, 0:31],
               start=False, stop=False, skip_group_check=True)
        for j in range(2):
            mm(out=ovs[j][:, :, 0:31], lhsT=wCM, rhs=xrs[j][:, :, 1:32],
               start=False, stop=False, skip_group_check=True)
        # --- K=64 quadrant group (2-col/cycle): rp-e, rm-o leftovers,
        # the row-boundary corrections for dense pass 2, and the rp
        # contribution for out-odd[15] that has no accA bank.
        mm(out=accB[u][0][0:64, 32:512], lhsT=wLQ[0:64, :], rhs=xv(u, 0, 480, 0, 64),
           start=False, stop=False, skip_group_check=True)
        mm(out=accB[u][1][0:64, 0:512], lhsT=wLQ[0:64, :], rhs=xv(u, 480, 512, 0, 64),
           start=False, stop=False, skip_group_check=True)
        mm(out=accB[u][0][64:128, 0:512], lhsT=wLQ[64:128, :],
           rhs=xv(u, 32, 512, 64, 128),
           start=False, stop=(u == 0), skip_group_check=True)
        mm(out=accB[u][1][64:128, 0:480], lhsT=wLQ[64:128, :],
           rhs=xv(u, 544, 480, 64, 128),
           start=False, stop=False, skip_group_check=True)
        if u < U - 1:  # rm-e row-31 correction: -Wrm . x_even[next plane row 0]
            mm(out=accB[u][1][0:64, 480:512], lhsT=wCR[0:64, :],
               rhs=xv(u + 1, 0, 32, 0, 64),
               start=False, stop=True, skip_group_check=True)
        if u > 0:      # rp-o row-0 correction: -Wrp . x_odd[prev plane row 31]
            mm(out=accB[u][0][64:128, 0:32], lhsT=wCR[64:128, :],
               rhs=xv(u - 1, 992, 32, 64, 128),
               start=False, stop=(u < U - 1), skip_group_check=True)
        if u == U - 1:  # rp for out-odd[15] rows 1-31 (accA has no bank here)
            mm(out=accB[u][0][64:128, 32:512], lhsT=wR15,
               rhs=xv(u, 0, 480, 64, 128),
               start=False, stop=True, skip_group_check=True)
            mm(out=accB[u][1][64:128, 0:512], lhsT=wR15,
               rhs=xv(u, 480, 512, 64, 128),
               start=False, stop=True, skip_group_check=True)

        if u >= 1:
            emit_evac(u - 1)
    emit_evac(U - 1)

    for p in (spool, apool, x2pool, xpool, wpool):
        p.release()


def _split_multi_waits(nc, mybir):
    """Walrus codegen allows only one sem-wait slot per engine instruction
    ("Too many sync wait commands").  Hoist all but one wait of any
    multi-wait instruction onto InstNoOp's inserted immediately before it
    on the same engine queue — semantically identical for in-order
    engines (the nop blocks the queue until its wait passes).
    """
    SyncInfo = mybir.SyncInfo
    counter = [0]
    for blk in nc.m.functions[0].blocks:
        insts = list(blk.instructions)
        out, changed = [], False
        for inst in insts:
            si = getattr(inst, "sync_info", None)
            waits = list(si.on_wait) if si is not None and si.on_wait else []
            if len(waits) > 1:
                for w in waits[:-1]:
                    nop = mybir.InstNoOp(name=f"waitnop_{counter[0]}")
                    counter[0] += 1
                    nop.engine = inst.engine
                    nop.sync_info = SyncInfo(on_wait=[w], on_update=[])
                    nc.register_instruction(nop, overwrite=True)
                    out.append(nop)
                si.on_wait = [waits[-1]]
                changed = True
            out.append(inst)
        if changed:
            blk.instructions = out


def build_nc():
    import concourse.bass as bass
    import concourse.mybir as mybir
    import concourse.tile as tile

    key = (tuple(CHUNKS), SG)
    if key in _NC_CACHE:
        return _NC_CACHE[key]
    nc = bass.Bass("TRN2", target_bir_lowering=False, debug=False)
    x = nc.dram_tensor("x", [128, NF], mybir.dt.bfloat16, kind="ExternalInput").ap()
    wt = nc.dram_tensor("wt", [128, WCOLS], mybir.dt.bfloat16,
                        kind="ExternalInput").ap()
    out = nc.dram_tensor("out", [128, NF], mybir.dt.bfloat16,
                         kind="ExternalOutput").ap()
    with tile.TileContext(nc) as tc:
        _emit(nc, tc, x, wt, out, mybir, bass)
    _split_multi_waits(nc, mybir)
    _NC_CACHE[key] = nc
    return nc


def host_weights(inputs):
    """Pack the 7 64x64 weights into the [128, 704] bf16 lhsT block layout.

    cols 0-127:   A  = [[Wself^T, Wtp^T], [Wtm^T, Wself^T]]   (dense pass 1)
    cols 128-255: wD = [[Wrm^T,  Wtm^T], [Wtp^T, Wrp^T]]      (dense pass 2)
    cols 256-511: diag2(Wcp), diag2(Wcm)
    cols 512-575: lo = Wrp^T (rp-e quadrant), hi = Wrm^T (rm-o quadrant)
    cols 576-639: lo = -Wrm^T, hi = -Wrp^T (pass-2 row-boundary corrections)
    cols 640-703: hi = Wrp^T (out-odd[15] rp quadrant)
    """
    W = {n: np.asarray(inputs[n], dtype=np.float32)
         for n in ("w_self", "w_tp", "w_tm", "w_rp", "w_rm", "w_cp", "w_cm")}
    wt = np.zeros((128, WCOLS), np.float32)
    wt[0:64, 0:64] = W["w_self"].T
    wt[64:128, 0:64] = W["w_tm"].T
    wt[0:64, 64:128] = W["w_tp"].T
    wt[64:128, 64:128] = W["w_self"].T
    wt[0:64, 128:192] = W["w_rm"].T
    wt[64:128, 128:192] = W["w_tp"].T
    wt[0:64, 192:256] = W["w_tm"].T
    wt[64:128, 192:256] = W["w_rp"].T
    for i, n in enumerate(("w_cp", "w_cm")):
        c0 = 256 + i * 128
        wt[0:64, c0:c0 + 64] = W[n].T
        wt[64:128, c0 + 64:c0 + 128] = W[n].T
    wt[0:64, 512:576] = W["w_rp"].T
    wt[64:128, 512:576] = W["w_rm"].T
    wt[0:64, 576:640] = -W["w_rm"].T
    wt[64:128, 576:640] = -W["w_rp"].T
    wt[64:128, 640:704] = W["w_rp"].T
    return wt.astype(ml_dtypes.bfloat16)


def host_x(inputs):
    """Per-batch packed bf16 images [128, 16384]: even planes on rows 0-63,
    odd planes on rows 64-127."""
    x = np.asarray(inputs["x"], dtype=np.float32)
    xs = []
    for b in range(B):
        xe = x[b][:, 0::2].reshape(CI, NF)
        xo = x[b][:, 1::2].reshape(CI, NF)
        xs.append(np.ascontiguousarray(
            np.concatenate([xe, xo], axis=0)).astype(ml_dtypes.bfloat16))
    return xs


def host_out(res):
    """Unpack per-core [128, 16384] bf16 results to [B, 64, 32, 32, 32] f32."""
    out = np.empty((B, CO, T, R, C), np.float32)
    for b in range(B):
        o = np.asarray(res[b]["out"]).astype(np.float32).reshape(2, CO, U, R, C)
        out[b, :, 0::2] = o[0]
        out[b, :, 1::2] = o[1]
    return out


def kernel(**inputs):
    from concourse.bass_utils import run_bass_kernel_spmd

    nc = build_nc()
    wt = host_weights(inputs)
    xs = host_x(inputs)
    in_maps = [{"x": xs[b], "wt": wt} for b in range(B)]
    res = run_bass_kernel_spmd(nc, in_maps, list(range(B))).results
    return host_out(res)


# revision 29
# speedup vs baseline: 1.3340x; 1.2606x over previous
"""DirectionalConv3d Trainium2 kernel — pack-T2 layout, bf16 I/O.

out[b, o, t, r, c] = sum_d W_d[o, :] . x[b, :, (t,r,c)+delta_d]
for the 7-point directional stencil (self, t+-1, r+-1, c+-1), zero padded.

Strategy (1 batch per core, 8 cores):
  - Host casts x to bf16 and packs plane-parity onto partition halves:
    partition p<64 holds channel p of EVEN t-planes, partition 64+p holds
    channel p of ODD t-planes ("superplane" u = plane pair (2u, 2u+1),
    free dim = u*1024 + r*32 + c).  Output uses the same packed layout
    (psum partitions 0-63 = out[2u] channels, 64-127 = out[2u+1]) and is
    written back as bf16; the host unpacks and casts to f32.
  - Per superplane, 7 stencil directions collapse into:
      * one dense K=128 "A" matmul  lhsT=[[Wself,Wtp],[Wtm,Wself]]
        covering self(both planes) + tp/tm INSIDE the pair at full
        16384-MAC/cycle array efficiency,
      * two K=64 off-diagonal quadrant matmuls for the cross-pair t terms
        (out[2u] += Wtp x[2u-1], out[2u+1] += Wtm x[2u+2]) which stream
        at 2 bf16 cols/cycle,
      * four K=128 block-diagonal matmuls diag(Wd, Wd) for r+-1 / c+-1
        (same spatial shift for both plane parities).  r shifts are
        contiguous-AP row trims; c shifts use 2-D strided APs
        [16 rows, 31 cols] so no padding and no wrap corrections exist.
    This halves PE column-issues vs the all-K=64 formulation and the
    bf16 I/O halves HBM traffic vs f32 (the two former co-bottlenecks).
  - PSUM: 2 banks per superplane, 8-bank rotation (4 superplanes in
    flight).  VectorE/ScalarE evacuate psum f32 -> bf16 staging, DMA out.
"""

import numpy as np
import ml_dtypes
import os

B = 8
CI = 64
CO = 64
T = 32
R = 32
C = 32
U = T // 2           # 16 superplanes
SPL = R * C          # 1024 elements per (super)plane per partition
NF = U * SPL         # 16384 free elements per partition half
WCOLS = 704          # weight SBUF columns: A | rp | rm | cp | cm | cross

SG = int(os.environ.get("KERNEL_SG", "2"))    # superplanes per output stage
# input chunk sizes in superplanes: small first chunks let the first
# matmul start as soon as ~256 KB has landed instead of ~1 MB.
CHUNKS = [int(c) for c in os.environ.get("KERNEL_CHUNKS", "1,1,2,4,4,4").split(",")]
assert sum(CHUNKS) == U
_CHUNK_OF = []
for _k, _c in enumerate(CHUNKS):
    _CHUNK_OF += [_k] * _c
_CHUNK_BASE = [sum(CHUNKS[:k]) for k in range(len(CHUNKS))]

# output stage groups: SG superplanes each, but the last two flush singly
_SGROUPS = []
_u = 0
while _u < U:
    _g1 = min(_u + SG, U) if _u < U - 2 else _u + 1
    _SGROUPS.append((_u, _g1))
    _u = _g1
_SGROUP_OF = {}
for _g0, _g1 in _SGROUPS:
    for _uu in range(_g0, _g1):
        _SGROUP_OF[_uu] = (_g0, _g1)

_NC_CACHE = {}


def _emit(nc, tc, x, wt, out, mybir, bass):
    xdt = mybir.dt.bfloat16
    AP = bass.AP

    wpool = tc.alloc_tile_pool(name="wp", bufs=1)
    xpool = tc.alloc_tile_pool(name="xin", bufs=1)
    apool = tc.alloc_tile_pool(name="accp", bufs=7, space="PSUM")
    wmpool = tc.alloc_tile_pool(name="wmp", bufs=1, space="PSUM")
    spool = tc.alloc_tile_pool(name="stg", bufs=4)

    # ---- weights [128, 704] prepacked host-side (see host_weights) ----
    w_sb = wpool.tile([128, WCOLS], xdt, name="w_sb")
    nc.sync.dma_start(out=w_sb[0:64, :], in_=AP(wt.tensor, 0, [[WCOLS, 64], [1, WCOLS]]))
    nc.sync.dma_start(out=w_sb[64:128, :],
                      in_=AP(wt.tensor, 64 * WCOLS, [[WCOLS, 64], [1, WCOLS]]))
    wA = w_sb[:, 0:128]
    wRP = w_sb[:, 128:256]
    wRM = w_sb[:, 256:384]
    wCP = w_sb[:, 384:512]
    wCM = w_sb[:, 512:640]
    wTPx = w_sb[64:128, 640:704]   # cross: out[2u] += Wtp x[2u-1]
    wTMx = w_sb[0:64, 640:704]     # cross: out[2u+1] += Wtm x[2u+2]

    # ---- x image: direct bf16 DMA, no staging, no casts, no memsets ----
    # two 64-partition DMAs per chunk (complementary SDMA engine sets);
    # lo half on SWDGE (gpsimd — its own descriptor path, dodges the
    # HWDGE rings that are busy with the runtime ACT-table preamble),
    # hi half on the SP HWDGE ring.  Descriptor generation runs on two
    # independent engines this way.
    xts = []
    for k, c in enumerate(CHUNKS):
        n = c * SPL
        off = _CHUNK_BASE[k] * SPL
        xt = xpool.tile([128, n], xdt, name=f"xc{k}")
        nc.gpsimd.dma_start(out=xt[0:64, :],
                            in_=AP(x.tensor, off, [[NF, 64], [1, n]]))
        nc.sync.dma_start(out=xt[64:128, :],
                          in_=AP(x.tensor, 64 * NF + off, [[NF, 64], [1, n]]))
        xts.append(xt)

    def xv(u, lo, sz, p0=0, p1=128):
        """SBUF AP for packed superplane u, free offset lo, length sz."""
        k = _CHUNK_OF[u]
        base = (u - _CHUNK_BASE[k]) * SPL
        return xts[k][p0:p1, base + lo:base + lo + sz]

    def xvr(u, j, p0=0, p1=128):
        """[p, 16 rows, 32 cols] view of bank j's rows of superplane u."""
        k = _CHUNK_OF[u]
        base = (u - _CHUNK_BASE[k]) * SPL + j * 512
        return xts[k][p0:p1, base:base + 512].rearrange("p (r c) -> p r c", c=C)

    mm = nc.tensor.matmul

    # ---- PE warm-up: dummy matmuls on a locally-memset tile keep the HAM
    # activity monitor busy while the x chunks stream in, so the real
    # matmuls start at 2.4 GHz instead of ramping from 1.2.  PE would
    # otherwise idle through the whole fill phase.  Deliberately NOT on
    # w_sb: a DMA dependency would gate the warm-up on semaphores that
    # fire ~12us in (measured), defeating the point.
    nwarm = int(os.environ.get("KERNEL_WARM", "56"))
    if nwarm:
        wsrc = wpool.tile([128, 128], xdt, name="wsrc")
        nc.vector.memset(wsrc[:, :], 0.0)
        wacc = wmpool.tile([128, 128], mybir.dt.float32, name="wacc")
        for i in range(nwarm):
            mm(out=wacc[:, :], lhsT=wsrc[:, :], rhs=wsrc[:, :],
               start=True, stop=True, skip_group_check=True)

    stage = None
    for u in range(U):
        accs = [apool.tile([128, 512], mybir.dt.float32, name=f"a{u}_{j}",
                           tag="acc") for j in range(2)]
        # --- all K=128 full-array matmuls first (no tile-geometry switch,
        # LDWEIGHTS hides in the background weight buffer), same-lhsT pairs
        # adjacent; then the two K=64 cross quadrants grouped at the end so
        # the 128<->64 reconfiguration is paid once per superplane.
        for j in range(2):  # A: self(both) + tp/tm internal (dense K=128)
            mm(out=accs[j][:, :], lhsT=wA, rhs=xv(u, j * 512, 512),
               start=True, stop=False, skip_group_check=True)
        # r+-1 (block-diagonal K=128, contiguous row-trimmed APs)
        mm(out=accs[0][:, 32:512], lhsT=wRP, rhs=xv(u, 0, 480),
           start=False, stop=False, skip_group_check=True)
        mm(out=accs[1][:, 0:512], lhsT=wRP, rhs=xv(u, 480, 512),
           start=False, stop=False, skip_group_check=True)
        mm(out=accs[0][:, 0:512], lhsT=wRM, rhs=xv(u, 32, 512),
           start=False, stop=False, skip_group_check=True)
        mm(out=accs[1][:, 0:480], lhsT=wRM, rhs=xv(u, 544, 480),
           start=False, stop=False, skip_group_check=True)
        # c+-1 (block-diagonal K=128, 2-D strided APs)
        ovs = [accs[j][:, :].rearrange("p (r c) -> p r c", c=C) for j in range(2)]
        xrs = [xvr(u, j) for j in range(2)]
        for j in range(2):
            mm(out=ovs[j][:, :, 1:32], lhsT=wCP, rhs=xrs[j][:, :, 0:31],
               start=False, stop=False, skip_group_check=True)
        for j in range(2):
            mm(out=ovs[j][:, :, 0:31], lhsT=wCM, rhs=xrs[j][:, :, 1:32],
               start=False, stop=False, skip_group_check=True)
        # cross-pair t terms (K=64 quadrants), stop on the last per bank
        for j in range(2):
            if u > 0:
                mm(out=accs[j][0:64, :], lhsT=wTPx,
                   rhs=xv(u - 1, j * 512, 512, 64, 128),
                   start=False, stop=(u == U - 1), skip_group_check=True)
        for j in range(2):
            if u < U - 1:
                mm(out=accs[j][64:128, :], lhsT=wTMx,
                   rhs=xv(u + 1, j * 512, 512, 0, 64),
                   start=False, stop=True, skip_group_check=True)

        # ---- evacuate PSUM -> bf16 staging; DMA out per stage group.
        # The last two superplanes flush individually so the final
        # (unoverlappable) DMA is as small as possible.
        g0, g1 = _SGROUP_OF[u]
        if u == g0:
            stage = spool.tile([128, (g1 - g0) * SPL], xdt, name=f"st{u}",
                               tag="st")
        soff = (u - g0) * SPL
        nc.vector.tensor_copy(out=stage[:, soff:soff + 512], in_=accs[0][:, :])
        nc.scalar.copy(out=stage[:, soff + 512:soff + SPL], in_=accs[1][:, :])
        if u == g1 - 1:
            # out goes on the ACT HWDGE ring: the SP ring is busy draining
            # input-hi chunks (FIFO per ring => head-of-line blocking), and
            # the SWDGE ring drains input-lo.
            n = (g1 - g0) * SPL
            nc.scalar.dma_start(
                out=AP(out.tensor, g0 * SPL, [[NF, 64], [1, n]]),
                in_=stage[0:64, :n])
            nc.scalar.dma_start(
                out=AP(out.tensor, 64 * NF + g0 * SPL, [[NF, 64], [1, n]]),
                in_=stage[64:128, :n])

    for p in (spool, wmpool, apool, xpool, wpool):
        p.release()


def _split_multi_waits(nc, mybir):
    """Walrus codegen allows only one sem-wait slot per engine instruction
    ("Too many sync wait commands").  Hoist all but one wait of any
    multi-wait instruction onto InstNoOp's inserted immediately before it
    on the same engine queue — semantically identical for in-order
    engines (the nop blocks the queue until its wait passes).
    """
    SyncInfo = mybir.SyncInfo
    counter = [0]
    for blk in nc.m.functions[0].blocks:
        insts = list(blk.instructions)
        out, changed = [], False
        for inst in insts:
            si = getattr(inst, "sync_info", None)
            waits = list(si.on_wait) if si is not None and si.on_wait else []
            if len(waits) > 1:
                for w in waits[:-1]:
                    nop = mybir.InstNoOp(name=f"waitnop_{counter[0]}")
                    counter[0] += 1
                    nop.engine = inst.engine
                    nop.sync_info = SyncInfo(on_wait=[w], on_update=[])
                    nc.register_instruction(nop, overwrite=True)
                    out.append(nop)
                si.on_wait = [waits[-1]]
                changed = True
            out.append(inst)
        if changed:
            blk.instructions = out


def build_nc():
    import concourse.bass as bass
    import concourse.mybir as mybir
    import concourse.tile as tile

    key = (tuple(CHUNKS), SG)
    if key in _NC_CACHE:
        return _NC_CACHE[key]
    nc = bass.Bass("TRN2", target_bir_lowering=False, debug=False)
    x = nc.dram_tensor("x", [128, NF], mybir.dt.bfloat16, kind="ExternalInput").ap()
    wt = nc.dram_tensor("wt", [128, WCOLS], mybir.dt.bfloat16,
                        kind="ExternalInput").ap()
    out = nc.dram_tensor("out", [128, NF], mybir.dt.bfloat16,
                         kind="ExternalOutput").ap()
    with tile.TileContext(nc) as tc:
        _emit(nc, tc, x, wt, out, mybir, bass)
    _split_multi_waits(nc, mybir)
    _NC_CACHE[key] = nc
    return nc


def host_weights(inputs):
    """Pack the 7 64x64 weights into the [128, 704] bf16 lhsT block layout.

    cols 0-127:  A = [[Wself^T, Wtp^T], [Wtm^T, Wself^T]]
    cols 128-639: diag2(Wrp), diag2(Wrm), diag2(Wcp), diag2(Wcm)
    cols 640-703: rows 64-127 = Wtp^T (tp cross), rows 0-63 = Wtm^T (tm cross)
    """
    W = {n: np.asarray(inputs[n], dtype=np.float32)
         for n in ("w_self", "w_tp", "w_tm", "w_rp", "w_rm", "w_cp", "w_cm")}
    wt = np.zeros((128, WCOLS), np.float32)
    wt[0:64, 0:64] = W["w_self"].T
    wt[64:128, 0:64] = W["w_tm"].T
    wt[0:64, 64:128] = W["w_tp"].T
    wt[64:128, 64:128] = W["w_self"].T
    for i, n in enumerate(("w_rp", "w_rm", "w_cp", "w_cm")):
        c0 = 128 + i * 128
        wt[0:64, c0:c0 + 64] = W[n].T
        wt[64:128, c0 + 64:c0 + 128] = W[n].T
    wt[64:128, 640:704] = W["w_tp"].T
    wt[0:64, 640:704] = W["w_tm"].T
    return wt.astype(ml_dtypes.bfloat16)


def host_x(inputs):
    """Per-batch packed bf16 images [128, 16384]: even planes on rows 0-63,
    odd planes on rows 64-127."""
    x = np.asarray(inputs["x"], dtype=np.float32)
    xs = []
    for b in range(B):
        xe = x[b][:, 0::2].reshape(CI, NF)
        xo = x[b][:, 1::2].reshape(CI, NF)
        xs.append(np.ascontiguousarray(
            np.concatenate([xe, xo], axis=0)).astype(ml_dtypes.bfloat16))
    return xs


def host_out(res):
    """Unpack per-core [128, 16384] bf16 results to [B, 64, 32, 32, 32] f32."""
    out = np.empty((B, CO, T, R, C), np.float32)
    for b in range(B):
        o = np.asarray(res[b]["out"]).astype(np.float32).reshape(2, CO, U, R, C)
        out[b, :, 0::2] = o[0]
        out[b, :, 1::2] = o[1]
    return out


def kernel(**inputs):
    from concourse.bass_utils import run_bass_kernel_spmd

    nc = build_nc()
    wt = host_weights(inputs)
    xs = host_x(inputs)
    in_maps = [{"x": xs[b], "wt": wt} for b in range(B)]
    res = run_bass_kernel_spmd(nc, in_maps, list(range(B))).results
    return host_out(res)


# revision 31
# speedup vs baseline: 1.3655x; 1.0236x over previous
"""DirectionalConv3d Trainium2 kernel — pack-T2 layout, bf16 I/O.

out[b, o, t, r, c] = sum_d W_d[o, :] . x[b, :, (t,r,c)+delta_d]
for the 7-point directional stencil (self, t+-1, r+-1, c+-1), zero padded.

Strategy (1 batch per core, 8 cores):
  - Host casts x to bf16 and packs plane-parity onto partition halves:
    partition p<64 holds channel p of EVEN t-planes, partition 64+p holds
    channel p of ODD t-planes ("superplane" u = plane pair (2u, 2u+1),
    free dim = u*1024 + r*32 + c).  Output uses the same packed layout
    (psum partitions 0-63 = out[2u] channels, 64-127 = out[2u+1]) and is
    written back as bf16; the host unpacks and casts to f32.
  - Per superplane, 7 stencil directions collapse into:
      * one dense K=128 "A" matmul  lhsT=[[Wself,Wtp],[Wtm,Wself]]
        covering self(both planes) + tp/tm INSIDE the pair at full
        16384-MAC/cycle array efficiency,
      * two K=64 off-diagonal quadrant matmuls for the cross-pair t terms
        (out[2u] += Wtp x[2u-1], out[2u+1] += Wtm x[2u+2]) which stream
        at 2 bf16 cols/cycle,
      * four K=128 block-diagonal matmuls diag(Wd, Wd) for r+-1 / c+-1
        (same spatial shift for both plane parities).  r shifts are
        contiguous-AP row trims; c shifts use 2-D strided APs
        [16 rows, 31 cols] so no padding and no wrap corrections exist.
    This halves PE column-issues vs the all-K=64 formulation and the
    bf16 I/O halves HBM traffic vs f32 (the two former co-bottlenecks).
  - PSUM: 2 banks per superplane, 8-bank rotation (4 superplanes in
    flight).  VectorE/ScalarE evacuate psum f32 -> bf16 staging, DMA out.
"""

import numpy as np
import ml_dtypes
import os

B = 8
CI = 64
CO = 64
T = 32
R = 32
C = 32
U = T // 2           # 16 superplanes
SPL = R * C          # 1024 elements per (super)plane per partition
NF = U * SPL         # 16384 free elements per partition half
WCOLS = 704          # weight SBUF columns: A | rp | rm | cp | cm | cross

SG = int(os.environ.get("KERNEL_SG", "2"))    # superplanes per output stage
# input chunk sizes in superplanes: small first chunks let the first
# matmul start as soon as ~256 KB has landed instead of ~1 MB.
CHUNKS = [int(c) for c in os.environ.get("KERNEL_CHUNKS", "1,1,2,4,4,4").split(",")]
assert sum(CHUNKS) == U
_CHUNK_OF = []
for _k, _c in enumerate(CHUNKS):
    _CHUNK_OF += [_k] * _c
_CHUNK_BASE = [sum(CHUNKS[:k]) for k in range(len(CHUNKS))]

# output stage groups: SG superplanes each, but the last two flush singly
_SGROUPS = []
_u = 0
while _u < U:
    _g1 = min(_u + SG, U) if _u < U - 2 else _u + 1
    _SGROUPS.append((_u, _g1))
    _u = _g1
_SGROUP_OF = {}
for _g0, _g1 in _SGROUPS:
    for _uu in range(_g0, _g1):
        _SGROUP_OF[_uu] = (_g0, _g1)

_NC_CACHE = {}


def _emit(nc, tc, x, wt, out, mybir, bass):
    xdt = mybir.dt.bfloat16
    AP = bass.AP

    wpool = tc.alloc_tile_pool(name="wp", bufs=1)
    xpool = tc.alloc_tile_pool(name="xin", bufs=1)
    apool = tc.alloc_tile_pool(name="accp", bufs=7, space="PSUM")
    wmpool = tc.alloc_tile_pool(name="wmp", bufs=1, space="PSUM")
    spool = tc.alloc_tile_pool(name="stg", bufs=4)

    # ---- weights [128, 704] prepacked host-side (see host_weights) ----
    w_sb = wpool.tile([128, WCOLS], xdt, name="w_sb")
    nc.sync.dma_start(out=w_sb[0:64, :], in_=AP(wt.tensor, 0, [[WCOLS, 64], [1, WCOLS]]))
    nc.sync.dma_start(out=w_sb[64:128, :],
                      in_=AP(wt.tensor, 64 * WCOLS, [[WCOLS, 64], [1, WCOLS]]))
    wA = w_sb[:, 0:128]
    wRP = w_sb[:, 128:256]
    wRM = w_sb[:, 256:384]
    wCP = w_sb[:, 384:512]
    wCM = w_sb[:, 512:640]
    wTPx = w_sb[64:128, 640:704]   # cross: out[2u] += Wtp x[2u-1]
    wTMx = w_sb[0:64, 640:704]     # cross: out[2u+1] += Wtm x[2u+2]

    # ---- x image: direct bf16 DMA, no staging, no casts, no memsets ----
    # two 64-partition DMAs per chunk (complementary SDMA engine sets);
    # lo half on SWDGE (gpsimd — its own descriptor path, dodges the
    # HWDGE rings that are busy with the runtime ACT-table preamble),
    # hi half on the SP HWDGE ring.  Descriptor generation runs on two
    # independent engines this way.
    xts = []
    for k, c in enumerate(CHUNKS):
        n = c * SPL
        off = _CHUNK_BASE[k] * SPL
        xt = xpool.tile([128, n], xdt, name=f"xc{k}")
        nc.gpsimd.dma_start(out=xt[0:64, :],
                            in_=AP(x.tensor, off, [[NF, 64], [1, n]]))
        nc.sync.dma_start(out=xt[64:128, :],
                          in_=AP(x.tensor, 64 * NF + off, [[NF, 64], [1, n]]))
        xts.append(xt)

    def xv(u, lo, sz, p0=0, p1=128):
        """SBUF AP for packed superplane u, free offset lo, length sz."""
        k = _CHUNK_OF[u]
        base = (u - _CHUNK_BASE[k]) * SPL
        return xts[k][p0:p1, base + lo:base + lo + sz]

    def xvr(u, j, p0=0, p1=128):
        """[p, 16 rows, 32 cols] view of bank j's rows of superplane u."""
        k = _CHUNK_OF[u]
        base = (u - _CHUNK_BASE[k]) * SPL + j * 512
        return xts[k][p0:p1, base:base + 512].rearrange("p (r c) -> p r c", c=C)

    mm = nc.tensor.matmul

    # ---- PE warm-up: dummy matmuls on a locally-memset tile keep the HAM
    # activity monitor busy while the x chunks stream in, so the real
    # matmuls start at 2.4 GHz instead of ramping from 1.2.  PE would
    # otherwise idle through the whole fill phase.  Deliberately NOT on
    # w_sb: a DMA dependency would gate the warm-up on semaphores that
    # fire ~12us in (measured), defeating the point.
    nwarm = int(os.environ.get("KERNEL_WARM", "36"))
    if nwarm:
        wsrc = wpool.tile([128, 128], xdt, name="wsrc")
        nc.vector.memset(wsrc[:, :], 0.0)
        wacc = wmpool.tile([128, 128], mybir.dt.float32, name="wacc")
        for i in range(nwarm):
            mm(out=wacc[:, :], lhsT=wsrc[:, :], rhs=wsrc[:, :],
               start=True, stop=True, skip_group_check=True)

    accs = {}
    stage_ref = [None]

    def emit_k128(u):
        """A + r/c block-diagonal passes (all full-array, no geometry
        switch, LDWEIGHTS hides in the background weight buffer)."""
        a = accs[u]
        for j in range(2):  # A: self(both) + tp/tm internal (dense K=128)
            mm(out=a[j][:, :], lhsT=wA, rhs=xv(u, j * 512, 512),
               start=True, stop=False, skip_group_check=True)
        # r+-1 (block-diagonal K=128, contiguous row-trimmed APs)
        mm(out=a[0][:, 32:512], lhsT=wRP, rhs=xv(u, 0, 480),
           start=False, stop=False, skip_group_check=True)
        mm(out=a[1][:, 0:512], lhsT=wRP, rhs=xv(u, 480, 512),
           start=False, stop=False, skip_group_check=True)
        mm(out=a[0][:, 0:512], lhsT=wRM, rhs=xv(u, 32, 512),
           start=False, stop=False, skip_group_check=True)
        mm(out=a[1][:, 0:480], lhsT=wRM, rhs=xv(u, 544, 480),
           start=False, stop=False, skip_group_check=True)
        # c+-1 (block-diagonal K=128, 2-D strided APs)
        ovs = [a[j][:, :].rearrange("p (r c) -> p r c", c=C) for j in range(2)]
        xrs = [xvr(u, j) for j in range(2)]
        for j in range(2):
            mm(out=ovs[j][:, :, 1:32], lhsT=wCP, rhs=xrs[j][:, :, 0:31],
               start=False, stop=False, skip_group_check=True)
        for j in range(2):
            mm(out=ovs[j][:, :, 0:31], lhsT=wCM, rhs=xrs[j][:, :, 1:32],
               start=False, stop=False, skip_group_check=True)

    def emit_k64(u):
        """Cross-pair t terms (K=64 quadrants), stop on the last per bank."""
        a = accs[u]
        for j in range(2):
            if u > 0:
                mm(out=a[j][0:64, :], lhsT=wTPx,
                   rhs=xv(u - 1, j * 512, 512, 64, 128),
                   start=False, stop=(u == U - 1), skip_group_check=True)
        for j in range(2):
            if u < U - 1:
                mm(out=a[j][64:128, :], lhsT=wTMx,
                   rhs=xv(u + 1, j * 512, 512, 0, 64),
                   start=False, stop=True, skip_group_check=True)

    def emit_evac(u):
        """PSUM -> bf16 staging; DMA out per stage group.  The last two
        superplanes flush individually so the final (unoverlappable) DMA
        is as small as possible.  The two output halves go on DIFFERENT
        HWDGE rings (lo on ACT, hi on SP) so they drain in parallel —
        each ring is strictly FIFO."""
        a = accs[u]
        g0, g1 = _SGROUP_OF[u]
        if u == g0:
            stage_ref[0] = spool.tile([128, (g1 - g0) * SPL], xdt,
                                      name=f"st{u}", tag="st")
        stage = stage_ref[0]
        soff = (u - g0) * SPL
        nc.vector.tensor_copy(out=stage[:, soff:soff + 512], in_=a[0][:, :])
        nc.scalar.copy(out=stage[:, soff + 512:soff + SPL], in_=a[1][:, :])
        if u == g1 - 1:
            n = (g1 - g0) * SPL
            nc.scalar.dma_start(
                out=AP(out.tensor, g0 * SPL, [[NF, 64], [1, n]]),
                in_=stage[0:64, :n])
            nc.sync.dma_start(
                out=AP(out.tensor, 64 * NF + g0 * SPL, [[NF, 64], [1, n]]),
                in_=stage[64:128, :n])

    # Superplanes run in PAIRS: [K128(u), K128(u+1), K64(u), K64(u+1)] so
    # the 128<->64 tile-geometry reconfiguration (~225 ns measured, both
    # directions) is paid once per pair instead of once per superplane.
    # The last two superplanes stay unpaired so their stop matmuls (and
    # hence the final evacuation + DMA) come as early as possible.
    groups = [(u, u + 1) for u in range(0, U - 2, 2)] + [(U - 2,), (U - 1,)]
    for grp in groups:
        for u in grp:
            accs[u] = [apool.tile([128, 512], mybir.dt.float32,
                                  name=f"a{u}_{j}", tag="acc")
                       for j in range(2)]
            emit_k128(u)
        for u in grp:
            emit_k64(u)
        for u in grp:
            emit_evac(u)

    for p in (spool, wmpool, apool, xpool, wpool):
        p.release()


def _split_multi_waits(nc, mybir):
    """Walrus codegen allows only one sem-wait slot per engine instruction
    ("Too many sync wait commands").  Hoist all but one wait of any
    multi-wait instruction onto InstNoOp's inserted immediately before it
    on the same engine queue — semantically identical for in-order
    engines (the nop blocks the queue until its wait passes).
    """
    SyncInfo = mybir.SyncInfo
    counter = [0]
    for blk in nc.m.functions[0].blocks:
        insts = list(blk.instructions)
        out, changed = [], False
        for inst in insts:
            si = getattr(inst, "sync_info", None)
            waits = list(si.on_wait) if si is not None and si.on_wait else []
            if len(waits) > 1:
                for w in waits[:-1]:
                    nop = mybir.InstNoOp(name=f"waitnop_{counter[0]}")
                    counter[0] += 1
                    nop.engine = inst.engine
                    nop.sync_info = SyncInfo(on_wait=[w], on_update=[])
                    nc.register_instruction(nop, overwrite=True)
                    out.append(nop)
                si.on_wait = [waits[-1]]
                changed = True
            out.append(inst)
        if changed:
            blk.instructions = out


def build_nc():
    import concourse.bass as bass
    import concourse.mybir as mybir
    import concourse.tile as tile

    key = (tuple(CHUNKS), SG)
    if key in _NC_CACHE:
        return _NC_CACHE[key]
    nc = bass.Bass("TRN2", target_bir_lowering=False, debug=False)
    x = nc.dram_tensor("x", [128, NF], mybir.dt.bfloat16, kind="ExternalInput").ap()
    wt = nc.dram_tensor("wt", [128, WCOLS], mybir.dt.bfloat16,
                        kind="ExternalInput").ap()
    out = nc.dram_tensor("out", [128, NF], mybir.dt.bfloat16,
                         kind="ExternalOutput").ap()
    with tile.TileContext(nc) as tc:
        _emit(nc, tc, x, wt, out, mybir, bass)
    _split_multi_waits(nc, mybir)
    _NC_CACHE[key] = nc
    return nc


def host_weights(inputs):
    """Pack the 7 64x64 weights into the [128, 704] bf16 lhsT block layout.

    cols 0-127:  A = [[Wself^T, Wtp^T], [Wtm^T, Wself^T]]
    cols 128-639: diag2(Wrp), diag2(Wrm), diag2(Wcp), diag2(Wcm)
    cols 640-703: rows 64-127 = Wtp^T (tp cross), rows 0-63 = Wtm^T (tm cross)
    """
    W = {n: np.asarray(inputs[n], dtype=np.float32)
         for n in ("w_self", "w_tp", "w_tm", "w_rp", "w_rm", "w_cp", "w_cm")}
    wt = np.zeros((128, WCOLS), np.float32)
    wt[0:64, 0:64] = W["w_self"].T
    wt[64:128, 0:64] = W["w_tm"].T
    wt[0:64, 64:128] = W["w_tp"].T
    wt[64:128, 64:128] = W["w_self"].T
    for i, n in enumerate(("w_rp", "w_rm", "w_cp", "w_cm")):
        c0 = 128 + i * 128
        wt[0:64, c0:c0 + 64] = W[n].T
        wt[64:128, c0 + 64:c0 + 128] = W[n].T
    wt[64:128, 640:704] = W["w_tp"].T
    wt[0:64, 640:704] = W["w_tm"].T
    return wt.astype(ml_dtypes.bfloat16)


def host_x(inputs):
    """Per-batch packed bf16 images [128, 16384]: even planes on rows 0-63,
    odd planes on rows 64-127."""
    x = np.asarray(inputs["x"], dtype=np.float32)
    xs = []
    for b in range(B):
        xe = x[b][:, 0::2].reshape(CI, NF)
        xo = x[b][:, 1::2].reshape(CI, NF)
        xs.append(np.ascontiguousarray(
            np.concatenate([xe, xo], axis=0)).astype(ml_dtypes.bfloat16))
    return xs


def host_out(res):
    """Unpack per-core [128, 16384] bf16 results to [B, 64, 32, 32, 32] f32."""
    out = np.empty((B, CO, T, R, C), np.float32)
    for b in range(B):
        o = np.asarray(res[b]["out"]).astype(np.float32).reshape(2, CO, U, R, C)
        out[b, :, 0::2] = o[0]
        out[b, :, 1::2] = o[1]
    return out


def kernel(**inputs):
    from concourse.bass_utils import run_bass_kernel_spmd

    nc = build_nc()
    wt = host_weights(inputs)
    xs = host_x(inputs)
    in_maps = [{"x": xs[b], "wt": wt} for b in range(B)]
    res = run_bass_kernel_spmd(nc, in_maps, list(range(B))).results
    return host_out(res)
